# revision 17
# baseline (speedup 1.0000x reference)
"""XNOR-Net++ 3x3 conv (sign(x) (*) sign(w) * alpha*beta*gamma) on 8 TRN2 NeuronCores.

Sharding: data-parallel over batch (32 -> 4 per core), weights/scales replicated.

Per core:
- binarize x and w on-device to fp8e4 (+-1 is exact; PSUM accumulates fp32 exactly)
- ONE width+height padded sign image per slot [128, 2, 58, 58] fp8 (two persistent
  ping-pong slots, borders zeroed once); the 3 kx taps are column offsets in the
  moving AP, so no shifted copies and no per-image memsets
- 3x3 conv = 9 accumulating DoubleRow matmuls per [128, 448] output tile
  (K=256 via input-channel-block pairing, 2 fp8 weights/PE cell)
- weights transposed on-device via PE transpose; pair dim step 128 B (%16==0)
- epilogue: single DVE mul with precomputed abg[p, ob, pix] = alpha*beta*gamma
  (alpha folded into the beta*gamma broadcast via K=1 matmuls)
- output batched per (image, ob) into SBUF, then one 1.6 MB contiguous DMA
"""

from contextlib import ExitStack

import numpy as np

import concourse.bacc as bacc
import concourse.bass as bass
import concourse.mybir as mybir
import concourse.tile as tile
from concourse import masks
from concourse.bass_utils import run_bass_kernel_spmd

N_CORES = 8
B, C, H, KS = 32, 256, 56, 3
P = 128
CB = C // P  # input-channel blocks (2)
OB = C // P  # output-channel blocks (2)
HP = H + 2   # padded image rows (58)
WP = H + 2   # padded image cols (58)
R = 8        # output rows per matmul tile
T = H // R   # row tiles per image (7)
NT = R * H   # moving free dim per matmul (448)
HW = H * H   # pixels per image (3136)

F32 = mybir.dt.float32
BF16 = mybir.dt.bfloat16
FP8 = mybir.dt.float8e4
DR = mybir.MatmulPerfMode.DoubleRow


def build_conv(tc, out_ap, x_ap, w_ap, a_ap, b_ap, g_ap, BL):
    nc = tc.nc
    with ExitStack() as ctx:
        const_pool = ctx.enter_context(tc.tile_pool(name="const", bufs=1))
        wpool = ctx.enter_context(tc.tile_pool(name="w", bufs=1))
        xpool = ctx.enter_context(tc.tile_pool(name="x", bufs=2))
        psumpool = ctx.enter_context(tc.tile_pool(name="psum", bufs=4, space="PSUM"))
        opool = ctx.enter_context(tc.tile_pool(name="o", bufs=4))

        ident = const_pool.tile([P, P], BF16, name="ident")
        masks.make_identity(nc, ident)

        # ---- weight DMAs first on the gpsimd queue (it has the earliest
        # first-byte latency, ~2.7us vs ~8.7us for SP); separate tiles per ob
        # so the ob0 transposes don't wait on the ob1 sign (region tracking)
        w_dram = w_ap.rearrange("(ob p) i ky kx -> p ob (i ky kx)", p=P)
        w_f32s = [
            wpool.tile([P, C * KS * KS], F32, name=f"w_f32_{ob}")
            for ob in range(OB)
        ]
        w_sgns = [
            wpool.tile([P, C * KS * KS], BF16, name=f"w_sgn_{ob}")
            for ob in range(OB)
        ]
        for ob in range(OB):
            nc.gpsimd.dma_start(w_f32s[ob], w_dram[:, ob])

        # tiny scale DMAs next on the same queue
        a_row = const_pool.tile([1, C], F32, name="a_row")
        nc.gpsimd.dma_start(a_row, a_ap.rearrange("c u v -> (u v) c"))
        b_t = const_pool.tile([1, H], F32, name="b_t")
        nc.gpsimd.dma_start(b_t, b_ap[0:1, :, 0])
        g_t = const_pool.tile([1, H], F32, name="g_t")
        nc.gpsimd.dma_start(g_t, g_ap[0:1, 0, :])
        ones_t = const_pool.tile([1, P], F32, name="ones_t")
        nc.gpsimd.memset(ones_t, 1.0)

        # ---- persistent padded sign-image slots; borders zeroed once ----
        imgs = [
            wpool.tile([P, CB, HP, WP], FP8, name=f"img{s}") for s in range(2)
        ]
        nc.gpsimd.memset(imgs[0], 0.0)
        nc.gpsimd.memset(imgs[1], 0.0)

        HROWS = H // 2  # 28
        x_v = x_ap.rearrange("b (cb p) h w -> b p cb (h w)", p=P)

        def emit_dma_half(b, h, x_t, dma_engine):
            rs, re = h * HROWS, (h + 1) * HROWS
            dma_engine.dma_start(
                x_t[:, :, rs * H : re * H], x_v[b][:, :, rs * H : re * H]
            )

        def emit_sign_half(b, h, x_t):
            im = imgs[b % 2]
            rs, re = h * HROWS, (h + 1) * HROWS
            nc.scalar.sign(
                im[:, :, 1 + rs : 1 + re, 1 : H + 1],
                x_t.rearrange("p cb (h w) -> p cb h w", h=H)[:, :, rs:re, :],
            )
            return im

        def emit_load(b, dma_engine):
            x_t = xpool.tile([P, CB, HW], F32, name="x_t")
            for h in range(2):
                emit_dma_half(b, h, x_t, dma_engine)
                im = emit_sign_half(b, h, x_t)
            return im

        # ---- image 0 + weight signs, interleaved per ob:
        # SP queue:  x0h1 dma, x0h2 dma
        # ACT queue: w0 sign, x0h1 sign, w1 sign, x0h2 sign
        x0_t = xpool.tile([P, CB, HW], F32, name="x_t")
        for ob in range(OB):
            nc.scalar.sign(w_sgns[ob], w_f32s[ob])
            emit_dma_half(0, ob, x0_t, nc.sync)
            im_cur = emit_sign_half(0, ob, x0_t)

        # wT2[i_low, tap, ob, cb, o] in fp8; pair dim cb has byte-step 128 (%16==0)
        # PSUM->SBUF copies on DVE so ACT stays free for the image signs.
        # PE order: transposes ob0 | tiny K=1 scale matmuls | transposes ob1,
        # so the tiny matmuls don't delay the first conv-feeding transposes.
        g_bcast = const_pool.tile([P, H], F32, name="g_bcast")
        abg = const_pool.tile([P, OB, HW], F32, name="abg")
        abg_v = abg.rearrange("p o (i j) -> p o i j", i=H)
        ab = const_pool.tile([P, OB, H], F32, name="ab")
        wT2 = wpool.tile([P, KS * KS, OB, CB, P], FP8, name="wT2")
        for ob in range(OB):
            w_view = w_sgns[ob].rearrange("p (i kk) -> p kk i", kk=KS * KS)
            for ib in range(CB):
                for kk in range(KS * KS):
                    pt = psumpool.tile([P, P], BF16, name="pt", tag="pt", bufs=3)
                    nc.tensor.transpose(
                        pt, w_view[:, kk, ib * P : (ib + 1) * P], ident
                    )
                    nc.vector.tensor_copy(wT2[:, kk, ob, ib, :], pt)
            if ob == 0:
                # K=1 matmuls: g_bcast[p, j] = gamma[j] (ones stationary) and
                # ab[p, o, i] = alpha[o*128+p] * beta[i] (alpha stationary)
                gp = psumpool.tile([P, H], F32, name="bgp", tag="bgp", bufs=1)
                nc.tensor.matmul(gp, ones_t, g_t[0:1, :], start=True, stop=True)
                nc.vector.tensor_copy(g_bcast, gp)
                for o2 in range(OB):
                    abp = psumpool.tile([P, H], F32, name="bgp", tag="bgp", bufs=1)
                    nc.tensor.matmul(
                        abp,
                        a_row[0:1, o2 * P : (o2 + 1) * P],
                        b_t[0:1, :],
                        start=True,
                        stop=True,
                    )
                    nc.vector.tensor_copy(ab[:, o2, :], abp)
            nc.vector.tensor_mul(
                abg_v[:, ob],
                ab[:, ob, :].unsqueeze(2).to_broadcast((P, H, H)),
                g_bcast.unsqueeze(1).to_broadcast((P, H, H)),
            )

        # ---- main loop over local batches ----
        out_v = out_ap.rearrange("b (ob p) h w -> b ob p (h w)", p=P)
        for b in range(BL):
            im = im_cur
            for ob in range(OB):
                o_t = opool.tile([P, HW], F32, name="o_t")
                for t in range(T):
                    ps = psumpool.tile([P, NT], F32, name="cps", tag="cps", bufs=4)
                    for kk in range(KS * KS):
                        ky, kx = divmod(kk, KS)
                        rhs = im[:, :, t * R + ky : t * R + ky + R, kx : kx + H]
                        nc.tensor.matmul(
                            ps,
                            wT2[:, kk, ob, :, :],
                            rhs,
                            start=(kk == 0),
                            stop=(kk == KS * KS - 1),
                            perf_mode=DR,
                        )
                    sl = slice(t * NT, (t + 1) * NT)
                    nc.vector.tensor_mul(o_t[:, sl], ps, abg[:, ob, sl])
                    if ob == 0 and t == 1 and b + 1 < BL:
                        # prefetch next image mid-stream: DMA issued from the
                        # (idle) gpsimd queue so neither the SP queue (startup
                        # w/x DMAs) nor the out DMAs can block it
                        im_cur = emit_load(b + 1, nc.gpsimd)
                    if t in (3, 5):
                        cs = slice(0, 4 * NT) if t == 3 else slice(4 * NT, 6 * NT)
                        nc.scalar.dma_start(out_v[b, ob][:, cs], o_t[:, cs])
                cs = slice(6 * NT, T * NT)
                nc.scalar.dma_start(out_v[b, ob][:, cs], o_t[:, cs])


def build_nc(BL):
    nc = bacc.Bacc("TRN2", target_bir_lowering=False, debug=False)
    x = nc.dram_tensor("x", [BL, C, H, H], F32, kind="ExternalInput")
    w = nc.dram_tensor("weight", [C, C, KS, KS], F32, kind="ExternalInput")
    a = nc.dram_tensor("alpha", [C, 1, 1], F32, kind="ExternalInput")
    be = nc.dram_tensor("beta", [1, H, 1], F32, kind="ExternalInput")
    g = nc.dram_tensor("gamma", [1, 1, H], F32, kind="ExternalInput")
    o = nc.dram_tensor("out", [BL, C, H, H], F32, kind="ExternalOutput")
    with tile.TileContext(nc) as tc:
        build_conv(tc, o.ap(), x.ap(), w.ap(), a.ap(), be.ap(), g.ap(), BL)
    nc.compile()
    return nc


_nc_cache = {}


def _get_nc(BL):
    if BL not in _nc_cache:
        _nc_cache[BL] = build_nc(BL)
    return _nc_cache[BL]


def kernel(x, weight, alpha, beta, gamma):
    x = np.ascontiguousarray(np.asarray(x, dtype=np.float32))
    weight = np.ascontiguousarray(np.asarray(weight, dtype=np.float32))
    alpha = np.ascontiguousarray(np.asarray(alpha, dtype=np.float32))
    beta = np.ascontiguousarray(np.asarray(beta, dtype=np.float32))
    gamma = np.ascontiguousarray(np.asarray(gamma, dtype=np.float32))

    BL = B // N_CORES
    nc = _get_nc(BL)
    xs = x.reshape(N_CORES, BL, C, H, H)
    in_maps = [
        {"x": xs[c], "weight": weight, "alpha": alpha, "beta": beta, "gamma": gamma}
        for c in range(N_CORES)
    ]
    res = run_bass_kernel_spmd(nc, in_maps, list(range(N_CORES)))
    return np.concatenate([r["out"] for r in res.results], axis=0)


# revision 18
# speedup vs baseline: 1.0020x; 1.0020x over previous
"""XNOR-Net++ 3x3 conv (sign(x) (*) sign(w) * alpha*beta*gamma) on 8 TRN2 NeuronCores.

Sharding: data-parallel over batch (32 -> 4 per core), weights/scales replicated.

Per core:
- binarize x and w on-device to fp8e4 (+-1 is exact; PSUM accumulates fp32 exactly)
- ONE width+height padded sign image per slot [128, 2, 58, 58] fp8 (two persistent
  ping-pong slots, borders zeroed once); the 3 kx taps are column offsets in the
  moving AP, so no shifted copies and no per-image memsets
- 3x3 conv = 9 accumulating DoubleRow matmuls per [128, 448] output tile
  (K=256 via input-channel-block pairing, 2 fp8 weights/PE cell)
- weights transposed on-device via PE transpose; pair dim step 128 B (%16==0)
- epilogue: single DVE mul with precomputed abg[p, ob, pix] = alpha*beta*gamma
  (alpha folded into the beta*gamma broadcast via K=1 matmuls)
- output batched per (image, ob) into SBUF, then one 1.6 MB contiguous DMA
"""

from contextlib import ExitStack

import numpy as np

import concourse.bacc as bacc
import concourse.bass as bass
import concourse.mybir as mybir
import concourse.tile as tile
from concourse import masks
from concourse.bass_utils import run_bass_kernel_spmd

N_CORES = 8
B, C, H, KS = 32, 256, 56, 3
P = 128
CB = C // P  # input-channel blocks (2)
OB = C // P  # output-channel blocks (2)
HP = H + 2   # padded image rows (58)
WP = H + 2   # padded image cols (58)
R = 8        # output rows per matmul tile
T = H // R   # row tiles per image (7)
NT = R * H   # moving free dim per matmul (448)
HW = H * H   # pixels per image (3136)

F32 = mybir.dt.float32
BF16 = mybir.dt.bfloat16
FP8 = mybir.dt.float8e4
DR = mybir.MatmulPerfMode.DoubleRow


def build_conv(tc, out_ap, x_ap, w_ap, a_ap, b_ap, g_ap, BL):
    nc = tc.nc
    with ExitStack() as ctx:
        const_pool = ctx.enter_context(tc.tile_pool(name="const", bufs=1))
        wpool = ctx.enter_context(tc.tile_pool(name="w", bufs=1))
        xpool = ctx.enter_context(tc.tile_pool(name="x", bufs=2))
        psumpool = ctx.enter_context(tc.tile_pool(name="psum", bufs=4, space="PSUM"))
        opool = ctx.enter_context(tc.tile_pool(name="o", bufs=4))

        # ---- weight DMAs first on the gpsimd queue (it has the earliest
        # first-byte latency, ~2.7us vs ~8.7us for SP); separate tiles per ob
        # so the ob0 transposes don't wait on the ob1 sign (region tracking)
        w_dram = w_ap.rearrange("(ob p) i ky kx -> p ob (i ky kx)", p=P)
        w_f32s = [
            wpool.tile([P, C * KS * KS], F32, name=f"w_f32_{ob}")
            for ob in range(OB)
        ]
        w_sgns = [
            wpool.tile([P, C * KS * KS], BF16, name=f"w_sgn_{ob}")
            for ob in range(OB)
        ]
        for ob in range(OB):
            nc.gpsimd.dma_start(w_f32s[ob], w_dram[:, ob])

        # tiny scale DMAs next on the same queue
        a_row = const_pool.tile([1, C], F32, name="a_row")
        nc.gpsimd.dma_start(a_row, a_ap.rearrange("c u v -> (u v) c"))
        b_t = const_pool.tile([1, H], F32, name="b_t")
        nc.gpsimd.dma_start(b_t, b_ap[0:1, :, 0])
        g_t = const_pool.tile([1, H], F32, name="g_t")
        nc.gpsimd.dma_start(g_t, g_ap[0:1, 0, :])
        ones_t = const_pool.tile([1, P], F32, name="ones_t")
        nc.gpsimd.memset(ones_t, 1.0)

        # ---- persistent padded sign-image slots; only the borders need
        # zeroing (the interior is overwritten by each image's sign) ----
        imgs = [
            wpool.tile([P, CB, HP, WP], FP8, name=f"img{s}") for s in range(2)
        ]

        def memset_border(im):
            nc.gpsimd.memset(im[:, :, 0:1, :], 0.0)
            nc.gpsimd.memset(im[:, :, HP - 1 : HP, :], 0.0)
            nc.gpsimd.memset(im[:, :, 1 : HP - 1, 0:1], 0.0)
            nc.gpsimd.memset(im[:, :, 1 : HP - 1, WP - 1 : WP], 0.0)

        memset_border(imgs[0])
        ident = const_pool.tile([P, P], BF16, name="ident")
        masks.make_identity(nc, ident)
        memset_border(imgs[1])

        HROWS = H // 2  # 28
        x_v = x_ap.rearrange("b (cb p) h w -> b p cb (h w)", p=P)

        def emit_dma_half(b, h, x_t, dma_engine):
            rs, re = h * HROWS, (h + 1) * HROWS
            dma_engine.dma_start(
                x_t[:, :, rs * H : re * H], x_v[b][:, :, rs * H : re * H]
            )

        def emit_sign_half(b, h, x_t):
            im = imgs[b % 2]
            rs, re = h * HROWS, (h + 1) * HROWS
            nc.scalar.sign(
                im[:, :, 1 + rs : 1 + re, 1 : H + 1],
                x_t.rearrange("p cb (h w) -> p cb h w", h=H)[:, :, rs:re, :],
            )
            return im

        def emit_load(b, dma_engine):
            x_t = xpool.tile([P, CB, HW], F32, name="x_t")
            for h in range(2):
                emit_dma_half(b, h, x_t, dma_engine)
                im = emit_sign_half(b, h, x_t)
            return im

        # ---- image 0 + weight signs, interleaved per ob:
        # SP queue:  x0h1 dma, x0h2 dma
        # ACT queue: w0 sign, x0h1 sign, w1 sign, x0h2 sign
        x0_t = xpool.tile([P, CB, HW], F32, name="x_t")
        for ob in range(OB):
            nc.scalar.sign(w_sgns[ob], w_f32s[ob])
            emit_dma_half(0, ob, x0_t, nc.sync)
            im_cur = emit_sign_half(0, ob, x0_t)

        # wT2[i_low, tap, ob, cb, o] in fp8; pair dim cb has byte-step 128 (%16==0)
        # PSUM->SBUF copies on DVE so ACT stays free for the image signs.
        # PE order: transposes ob0 | tiny K=1 scale matmuls | transposes ob1,
        # so the tiny matmuls don't delay the first conv-feeding transposes.
        g_bcast = const_pool.tile([P, H], F32, name="g_bcast")
        abg = const_pool.tile([P, OB, HW], F32, name="abg")
        abg_v = abg.rearrange("p o (i j) -> p o i j", i=H)
        ab = const_pool.tile([P, OB, H], F32, name="ab")
        wT2 = wpool.tile([P, KS * KS, OB, CB, P], FP8, name="wT2")
        for ob in range(OB):
            w_view = w_sgns[ob].rearrange("p (i kk) -> p kk i", kk=KS * KS)
            for ib in range(CB):
                for kk in range(KS * KS):
                    pt = psumpool.tile([P, P], BF16, name="pt", tag="pt", bufs=3)
                    nc.tensor.transpose(
                        pt, w_view[:, kk, ib * P : (ib + 1) * P], ident
                    )
                    nc.vector.tensor_copy(wT2[:, kk, ob, ib, :], pt)
            if ob == 0:
                # K=1 matmuls: g_bcast[p, j] = gamma[j] (ones stationary) and
                # ab[p, o, i] = alpha[o*128+p] * beta[i] (alpha stationary)
                gp = psumpool.tile([P, H], F32, name="bgp", tag="bgp", bufs=1)
                nc.tensor.matmul(gp, ones_t, g_t[0:1, :], start=True, stop=True)
                nc.vector.tensor_copy(g_bcast, gp)
                for o2 in range(OB):
                    abp = psumpool.tile([P, H], F32, name="bgp", tag="bgp", bufs=1)
                    nc.tensor.matmul(
                        abp,
                        a_row[0:1, o2 * P : (o2 + 1) * P],
                        b_t[0:1, :],
                        start=True,
                        stop=True,
                    )
                    nc.vector.tensor_copy(ab[:, o2, :], abp)
            nc.vector.tensor_mul(
                abg_v[:, ob],
                ab[:, ob, :].unsqueeze(2).to_broadcast((P, H, H)),
                g_bcast.unsqueeze(1).to_broadcast((P, H, H)),
            )

        # ---- main loop over local batches ----
        out_v = out_ap.rearrange("b (ob p) h w -> b ob p (h w)", p=P)
        for b in range(BL):
            im = im_cur
            for ob in range(OB):
                o_t = opool.tile([P, HW], F32, name="o_t")
                for t in range(T):
                    ps = psumpool.tile([P, NT], F32, name="cps", tag="cps", bufs=4)
                    for kk in range(KS * KS):
                        ky, kx = divmod(kk, KS)
                        rhs = im[:, :, t * R + ky : t * R + ky + R, kx : kx + H]
                        nc.tensor.matmul(
                            ps,
                            wT2[:, kk, ob, :, :],
                            rhs,
                            start=(kk == 0),
                            stop=(kk == KS * KS - 1),
                            perf_mode=DR,
                        )
                    sl = slice(t * NT, (t + 1) * NT)
                    nc.vector.tensor_mul(o_t[:, sl], ps, abg[:, ob, sl])
                    if ob == 0 and t == 1 and b + 1 < BL:
                        # prefetch next image mid-stream: DMA issued from the
                        # (idle) gpsimd queue so neither the SP queue (startup
                        # w/x DMAs) nor the out DMAs can block it
                        im_cur = emit_load(b + 1, nc.gpsimd)
                    if t in (3, 5):
                        cs = slice(0, 4 * NT) if t == 3 else slice(4 * NT, 6 * NT)
                        nc.scalar.dma_start(out_v[b, ob][:, cs], o_t[:, cs])
                cs = slice(6 * NT, T * NT)
                nc.scalar.dma_start(out_v[b, ob][:, cs], o_t[:, cs])


def build_nc(BL):
    nc = bacc.Bacc("TRN2", target_bir_lowering=False, debug=False)
    x = nc.dram_tensor("x", [BL, C, H, H], F32, kind="ExternalInput")
    w = nc.dram_tensor("weight", [C, C, KS, KS], F32, kind="ExternalInput")
    a = nc.dram_tensor("alpha", [C, 1, 1], F32, kind="ExternalInput")
    be = nc.dram_tensor("beta", [1, H, 1], F32, kind="ExternalInput")
    g = nc.dram_tensor("gamma", [1, 1, H], F32, kind="ExternalInput")
    o = nc.dram_tensor("out", [BL, C, H, H], F32, kind="ExternalOutput")
    with tile.TileContext(nc) as tc:
        build_conv(tc, o.ap(), x.ap(), w.ap(), a.ap(), be.ap(), g.ap(), BL)
    nc.compile()
    return nc


_nc_cache = {}


def _get_nc(BL):
    if BL not in _nc_cache:
        _nc_cache[BL] = build_nc(BL)
    return _nc_cache[BL]


def kernel(x, weight, alpha, beta, gamma):
    x = np.ascontiguousarray(np.asarray(x, dtype=np.float32))
    weight = np.ascontiguousarray(np.asarray(weight, dtype=np.float32))
    alpha = np.ascontiguousarray(np.asarray(alpha, dtype=np.float32))
    beta = np.ascontiguousarray(np.asarray(beta, dtype=np.float32))
    gamma = np.ascontiguousarray(np.asarray(gamma, dtype=np.float32))

    BL = B // N_CORES
    nc = _get_nc(BL)
    xs = x.reshape(N_CORES, BL, C, H, H)
    in_maps = [
        {"x": xs[c], "weight": weight, "alpha": alpha, "beta": beta, "gamma": gamma}
        for c in range(N_CORES)
    ]
    res = run_bass_kernel_spmd(nc, in_maps, list(range(N_CORES)))
    return np.concatenate([r["out"] for r in res.results], axis=0)


# revision 19
# speedup vs baseline: 1.0898x; 1.0877x over previous
"""XNOR-Net++ 3x3 conv (sign(x) (*) sign(w) * alpha*beta*gamma) on 8 TRN2 NeuronCores.

Sharding: data-parallel over batch (32 -> 4 per core), weights/scales replicated.

Per core:
- binarize x and w on-device to fp8e4 (+-1 is exact; PSUM accumulates fp32 exactly)
- ONE width+height padded sign image per slot [128, 2, 58, 58] fp8 (two persistent
  ping-pong slots, borders zeroed once); the 3 kx taps are column offsets in the
  moving AP, so no shifted copies and no per-image memsets
- 3x3 conv = 9 accumulating DoubleRow matmuls per [128, 448] output tile
  (K=256 via input-channel-block pairing, 2 fp8 weights/PE cell)
- weights transposed on-device via PE transpose; pair dim step 128 B (%16==0)
- epilogue: single DVE mul with precomputed abg[p, ob, pix] = alpha*beta*gamma
  (alpha folded into the beta*gamma broadcast via K=1 matmuls)
- output batched per (image, ob) into SBUF, then one 1.6 MB contiguous DMA
"""

from contextlib import ExitStack

import numpy as np

import concourse.bacc as bacc
import concourse.bass as bass
import concourse.mybir as mybir
import concourse.tile as tile
from concourse import masks
from concourse.bass_utils import run_bass_kernel_spmd

N_CORES = 8
B, C, H, KS = 32, 256, 56, 3
P = 128
CB = C // P  # input-channel blocks (2)
OB = C // P  # output-channel blocks (2)
HP = H + 2   # padded image rows (58)
WP = H + 2   # padded image cols (58)
R = 8        # output rows per matmul tile
T = H // R   # row tiles per image (7)
NT = R * H   # moving free dim per matmul (448)
HW = H * H   # pixels per image (3136)

F32 = mybir.dt.float32
BF16 = mybir.dt.bfloat16
FP8 = mybir.dt.float8e4
DR = mybir.MatmulPerfMode.DoubleRow


def build_conv(tc, out_ap, x_ap, w_ap, a_ap, b_ap, g_ap, BL):
    nc = tc.nc
    with ExitStack() as ctx:
        const_pool = ctx.enter_context(tc.tile_pool(name="const", bufs=1))
        wpool = ctx.enter_context(tc.tile_pool(name="w", bufs=1))
        xpool = ctx.enter_context(tc.tile_pool(name="x", bufs=2))
        psumpool = ctx.enter_context(tc.tile_pool(name="psum", bufs=4, space="PSUM"))
        opool = ctx.enter_context(tc.tile_pool(name="o", bufs=4))

        # ---- weight DMAs first on the gpsimd queue (it has the earliest
        # first-byte latency, ~2.7us vs ~8.7us for SP); separate tiles per ob
        # so the ob0 transposes don't wait on the ob1 sign (region tracking)
        w_dram = w_ap.rearrange("(ob p) i ky kx -> p ob (i ky kx)", p=P)
        w_f32s = [
            wpool.tile([P, C * KS * KS], F32, name=f"w_f32_{ob}")
            for ob in range(OB)
        ]
        w_sgns = [
            wpool.tile([P, C * KS * KS], BF16, name=f"w_sgn_{ob}")
            for ob in range(OB)
        ]
        # tiny scale DMAs on the gpsimd queue (lands by ~6us)
        a_row = const_pool.tile([1, C], F32, name="a_row")
        nc.gpsimd.dma_start(a_row, a_ap.rearrange("c u v -> (u v) c"))
        b_t = const_pool.tile([1, H], F32, name="b_t")
        nc.gpsimd.dma_start(b_t, b_ap[0:1, :, 0])
        g_t = const_pool.tile([1, H], F32, name="g_t")
        nc.gpsimd.dma_start(g_t, g_ap[0:1, 0, :])
        ones_t = const_pool.tile([1, P], F32, name="ones_t")
        nc.gpsimd.memset(ones_t, 1.0)

        # ---- persistent padded sign-image slots; only the borders need
        # zeroing (the interior is overwritten by each image's sign) ----
        imgs = [
            wpool.tile([P, CB, HP, WP], FP8, name=f"img{s}") for s in range(2)
        ]

        def memset_border(im):
            nc.gpsimd.memset(im[:, :, 0:1, :], 0.0)
            nc.gpsimd.memset(im[:, :, HP - 1 : HP, :], 0.0)
            nc.gpsimd.memset(im[:, :, 1 : HP - 1, 0:1], 0.0)
            nc.gpsimd.memset(im[:, :, 1 : HP - 1, WP - 1 : WP], 0.0)

        memset_border(imgs[0])
        ident = const_pool.tile([P, P], BF16, name="ident")
        masks.make_identity(nc, ident)
        memset_border(imgs[1])

        HROWS = H // 2  # 28
        x_v = x_ap.rearrange("b (cb p) h w -> b p cb (h w)", p=P)

        def emit_dma_half(b, h, x_t, dma_engine):
            rs, re = h * HROWS, (h + 1) * HROWS
            dma_engine.dma_start(
                x_t[:, :, rs * H : re * H], x_v[b][:, :, rs * H : re * H]
            )

        def emit_sign_half(b, h, x_t):
            im = imgs[b % 2]
            rs, re = h * HROWS, (h + 1) * HROWS
            nc.scalar.sign(
                im[:, :, 1 + rs : 1 + re, 1 : H + 1],
                x_t.rearrange("p cb (h w) -> p cb h w", h=H)[:, :, rs:re, :],
            )
            return im

        def emit_load(b, dma_engine):
            x_t = xpool.tile([P, CB, HW], F32, name="x_t")
            for h in range(2):
                emit_dma_half(b, h, x_t, dma_engine)
                im = emit_sign_half(b, h, x_t)
            return im

        # ---- weights + image 0, interleaved per ob:
        # SP queue (HWDGE): w0 dma, x0h1 dma, w1 dma, x0h2 dma
        # ACT queue: w0 sign, x0h1 sign, w1 sign, x0h2 sign
        x0_t = xpool.tile([P, CB, HW], F32, name="x_t")
        for ob in range(OB):
            nc.sync.dma_start(w_f32s[ob], w_dram[:, ob])
            nc.scalar.sign(w_sgns[ob], w_f32s[ob])
            emit_dma_half(0, ob, x0_t, nc.sync)
            im_cur = emit_sign_half(0, ob, x0_t)

        # wT2[i_low, tap, ob, cb, o] in fp8; pair dim cb has byte-step 128 (%16==0)
        # PSUM->SBUF copies on DVE so ACT stays free for the image signs.
        # PE order: transposes ob0 | tiny K=1 scale matmuls | transposes ob1,
        # so the tiny matmuls don't delay the first conv-feeding transposes.
        g_bcast = const_pool.tile([P, H], F32, name="g_bcast")
        abg = const_pool.tile([P, OB, HW], F32, name="abg")
        abg_v = abg.rearrange("p o (i j) -> p o i j", i=H)
        ab = const_pool.tile([P, OB, H], F32, name="ab")
        wT2 = wpool.tile([P, KS * KS, OB, CB, P], FP8, name="wT2")
        for ob in range(OB):
            w_view = w_sgns[ob].rearrange("p (i kk) -> p kk i", kk=KS * KS)
            for ib in range(CB):
                for kk in range(KS * KS):
                    pt = psumpool.tile([P, P], BF16, name="pt", tag="pt", bufs=3)
                    nc.tensor.transpose(
                        pt, w_view[:, kk, ib * P : (ib + 1) * P], ident
                    )
                    nc.vector.tensor_copy(wT2[:, kk, ob, ib, :], pt)
            if ob == 0:
                # K=1 matmuls: g_bcast[p, j] = gamma[j] (ones stationary) and
                # ab[p, o, i] = alpha[o*128+p] * beta[i] (alpha stationary)
                gp = psumpool.tile([P, H], F32, name="bgp", tag="bgp", bufs=1)
                nc.tensor.matmul(gp, ones_t, g_t[0:1, :], start=True, stop=True)
                nc.vector.tensor_copy(g_bcast, gp)
                for o2 in range(OB):
                    abp = psumpool.tile([P, H], F32, name="bgp", tag="bgp", bufs=1)
                    nc.tensor.matmul(
                        abp,
                        a_row[0:1, o2 * P : (o2 + 1) * P],
                        b_t[0:1, :],
                        start=True,
                        stop=True,
                    )
                    nc.vector.tensor_copy(ab[:, o2, :], abp)
            nc.vector.tensor_mul(
                abg_v[:, ob],
                ab[:, ob, :].unsqueeze(2).to_broadcast((P, H, H)),
                g_bcast.unsqueeze(1).to_broadcast((P, H, H)),
            )

        # ---- main loop over local batches ----
        out_v = out_ap.rearrange("b (ob p) h w -> b ob p (h w)", p=P)
        for b in range(BL):
            im = im_cur
            for ob in range(OB):
                o_t = opool.tile([P, HW], F32, name="o_t")
                for t in range(T):
                    ps = psumpool.tile([P, NT], F32, name="cps", tag="cps", bufs=4)
                    for kk in range(KS * KS):
                        ky, kx = divmod(kk, KS)
                        rhs = im[:, :, t * R + ky : t * R + ky + R, kx : kx + H]
                        nc.tensor.matmul(
                            ps,
                            wT2[:, kk, ob, :, :],
                            rhs,
                            start=(kk == 0),
                            stop=(kk == KS * KS - 1),
                            perf_mode=DR,
                        )
                    sl = slice(t * NT, (t + 1) * NT)
                    nc.vector.tensor_mul(o_t[:, sl], ps, abg[:, ob, sl])
                    if ob == 0 and t == 1 and b + 1 < BL:
                        # prefetch next image mid-stream: DMA issued from the
                        # ACT queue, whose program order delays the doorbell
                        # until after the previous image's signs (no bandwidth
                        # stolen from the startup w/x0 DMAs)
                        im_cur = emit_load(b + 1, nc.scalar)
                    if t in (3, 5):
                        cs = slice(0, 4 * NT) if t == 3 else slice(4 * NT, 6 * NT)
                        nc.scalar.dma_start(out_v[b, ob][:, cs], o_t[:, cs])
                cs = slice(6 * NT, T * NT)
                nc.scalar.dma_start(out_v[b, ob][:, cs], o_t[:, cs])


def build_nc(BL):
    nc = bacc.Bacc("TRN2", target_bir_lowering=False, debug=False)
    x = nc.dram_tensor("x", [BL, C, H, H], F32, kind="ExternalInput")
    w = nc.dram_tensor("weight", [C, C, KS, KS], F32, kind="ExternalInput")
    a = nc.dram_tensor("alpha", [C, 1, 1], F32, kind="ExternalInput")
    be = nc.dram_tensor("beta", [1, H, 1], F32, kind="ExternalInput")
    g = nc.dram_tensor("gamma", [1, 1, H], F32, kind="ExternalInput")
    o = nc.dram_tensor("out", [BL, C, H, H], F32, kind="ExternalOutput")
    with tile.TileContext(nc) as tc:
        build_conv(tc, o.ap(), x.ap(), w.ap(), a.ap(), be.ap(), g.ap(), BL)
    nc.compile()
    return nc


_nc_cache = {}


def _get_nc(BL):
    if BL not in _nc_cache:
        _nc_cache[BL] = build_nc(BL)
    return _nc_cache[BL]


def kernel(x, weight, alpha, beta, gamma):
    x = np.ascontiguousarray(np.asarray(x, dtype=np.float32))
    weight = np.ascontiguousarray(np.asarray(weight, dtype=np.float32))
    alpha = np.ascontiguousarray(np.asarray(alpha, dtype=np.float32))
    beta = np.ascontiguousarray(np.asarray(beta, dtype=np.float32))
    gamma = np.ascontiguousarray(np.asarray(gamma, dtype=np.float32))

    BL = B // N_CORES
    nc = _get_nc(BL)
    xs = x.reshape(N_CORES, BL, C, H, H)
    in_maps = [
        {"x": xs[c], "weight": weight, "alpha": alpha, "beta": beta, "gamma": gamma}
        for c in range(N_CORES)
    ]
    res = run_bass_kernel_spmd(nc, in_maps, list(range(N_CORES)))
    return np.concatenate([r["out"] for r in res.results], axis=0)
